# revision 38
# baseline (speedup 1.0000x reference)
"""Causal self-attention (B=4, T=2048, C=1024, H=16) on 8 trn2 NeuronCores.

Sharding: tensor-parallel over heads. Core c owns heads (2c, 2c+1).
Each core computes QKV projection for its 2 heads (full x), causal
attention for its (4 batches x 2 heads), and a partial output projection
with its 128 rows of W_proj. Host sums the 8 bf16 partial outputs
(+ b_proj) in fp32.

Phase-merged schedule: only the first two t-pair QKV chains (batch 0's
tokens) run as a prologue; the remaining six (tps 2-7) and their V
transposes are folded into the attention loop's filler queue. The PE
then always has dense matmul work between score groups (instead of
idling on ACT latency), which both raises overlap and keeps the HAM
clock gate at K=8/8 (half-clock oscillation cost ~50us in the phased
schedule). Deadlines force any remaining QKV/transpose items before the
first chunk of the batch that needs them.

Device-side layout choices:
  - x is transposed host-side; loaded as [c_in, t] tiles (bf16).
  - Q,K produced transposed: [c_out(=2*64), t]; the two heads live on
    partition halves 0:64 / 64:128, so the two heads' score matmuls
    row-pack (concurrent row-tiled MMs) in the PE array.
  - scores S_T [k, q] per 128-k-block, diagonal blocks truncated to
    their live column range; exp on ACT (scale folded); causal triangle
    via bf16 mask multiplies on DVE.
  - softmax denominators ride the AV matmul as a 'ones' column of V1;
    reciprocal on DVE; broadcast over partitions via GPSIMD
    partition_broadcast (no PE involvement).
  - AV accumulates y_T unnormalized; normalization fused into the
    PSUM->SBUF evacuation multiply, emitted as soon as the previous
    chunk's AV fillers drain (the 2-deep always-'yps' PSUM ring needs
    the early release).
  - PSUM tags: sps pairs on 'sgroup' (4 banks), AV accumulators always
    on 'yps' (2 banks), QKV chains + PE transposes on 'bank' (2 banks).
  - projection interleaved chunk-by-chunk into the attention loop;
    chunk order is batch-major with batch 3 reversed (ends on a 4-block
    chunk), and tail projections alternate scalar/vector evacuation.
"""

import sys

sys.path.insert(0, "/opt/trn_rl_repo")

import numpy as np
import ml_dtypes

B, T, C, H = 4, 2048, 1024, 16
HD = C // H  # 64
BT = B * T  # 8192
NCORES = 8
TCH = 512  # t-chunk
NT = BT // TCH  # 16
NCC = C // 128  # 8 c_in chunks
KB = 128  # k block

_RUNNER = None


def _build_nc():
    import concourse.bacc as bacc
    import concourse.mybir as mybir
    import concourse.tile as tile
    from concourse.masks import make_identity

    f32 = mybir.dt.float32
    bf16 = mybir.dt.bfloat16
    Exp = mybir.ActivationFunctionType.Exp

    nc = bacc.Bacc(None, target_bir_lowering=False, debug=False)

    xtiles = nc.dram_tensor("xtiles", [NT, 128, NCC, TCH], bf16, kind="ExternalInput")
    wqkv = nc.dram_tensor("wqkv", [C, 384], bf16, kind="ExternalInput")
    bqkv = nc.dram_tensor("bqkv", [384], f32, kind="ExternalInput")
    wproj = nc.dram_tensor("wproj", [128, C], bf16, kind="ExternalInput")
    trimask = nc.dram_tensor("trimask", [128, 128], bf16, kind="ExternalInput")
    out_d = nc.dram_tensor("out", [BT, C], bf16, kind="ExternalOutput")

    with tile.TileContext(nc) as tc:
        with (
            tc.tile_pool(name="const", bufs=1) as const_pool,
            tc.tile_pool(name="big", bufs=1) as big_pool,
            tc.tile_pool(name="sb", bufs=2) as sb_pool,
            tc.tile_pool(name="ps", bufs=1, space="PSUM") as ps_pool,
        ):
            # --- constants ---
            wqkv_sb = const_pool.tile([128, NCC, 384], bf16)
            bias_sb = const_pool.tile([128, 3], f32)
            ident_sb = const_pool.tile([128, 128], bf16)
            make_identity(nc, ident_sb)
            wproj_sb = const_pool.tile([128, C], bf16)
            tri_sb = const_pool.tile([128, 128], bf16)

            # --- persistent activations ---
            qt_sb = big_pool.tile([128, BT], bf16)  # Q_T [2*64, t]
            kt_sb = big_pool.tile([128, BT], bf16)  # K_T
            # V1 per (t-block, head): cols 0:64 V_h, col 64 ones, pad to 80
            v1_sb = big_pool.tile([128, BT // 128, 2, 80], bf16)
            yt_sb = big_pool.tile([128, BT], bf16)  # y_T [c, t] normalized
            # (v1 memset is emitted after the prologue DMA issues so the
            # gpsimd queue's startup DMAs go out first)

            # ---------------- QKV building blocks ----------------
            def emit_vt1(vtmp, tb, j):
                vtp = ps_pool.tile(
                    [128, 128], bf16, tag="bank", bufs=2,
                    name=f"vtp_{tb}",
                )
                nc.tensor.transpose(
                    vtp, vtmp[:, j * 128 : (j + 1) * 128], ident_sb
                )
                # split rows (=heads) into V1 [128,2,64]
                dst = v1_sb[:, tb, :, 0:64]
                src = vtp.rearrange("p (h d) -> p h d", h=2)
                if j % 2 == 0:
                    nc.scalar.copy(dst, src)
                else:
                    nc.vector.tensor_copy(dst, src)

            def fetch_xt(tch, split=False):
                xt = sb_pool.tile(
                    [128, NCC, TCH], bf16, tag="xt", bufs=4,
                    name=f"xt_{tch}",
                )
                if split:
                    # first tile lands cc-pair by cc-pair so the first
                    # QKV chain starts earlier
                    for ccp in range(NCC // 2):
                        nc.sync.dma_start(
                            xt[:, 2 * ccp : 2 * ccp + 2, :],
                            xtiles.ap()[tch][:, 2 * ccp : 2 * ccp + 2, :],
                        )
                else:
                    nc.sync.dma_start(xt, xtiles.ap()[tch])
                return xt

            xt_pairs = {}  # tp -> [xt, xt]
            extra_fill = []  # deque of ("qkv", tp, o3) / ("vt", vtmp, tb, j)

            def emit_qkv(tp, o3, u, xts):
                # one output block (q, k or v cols) for ONE t-chunk: a
                # ~1.7us half-chain, fine-grained enough for the drain
                # scheduler to place accurately between score groups
                if o3 == 0 and u == 0 and tp + 1 < NT // 2 \
                        and tp + 1 not in xt_pairs:
                    xt_pairs[tp + 1] = [
                        fetch_xt(2 * tp + 2), fetch_xt(2 * tp + 3)
                    ]
                ps = ps_pool.tile(
                    [128, TCH], f32, tag="bank", bufs=2,
                    name=f"qkv_{tp}_{o3}_{u}",
                )
                for cc in range(NCC):
                    nc.tensor.matmul(
                        ps,
                        lhsT=wqkv_sb[:, cc, o3 * 128 : (o3 + 1) * 128],
                        rhs=xts[u][:, cc, :],
                        start=(cc == 0),
                        stop=(cc == NCC - 1),
                    )
                t0 = (2 * tp + u) * TCH
                if o3 == 0:
                    nc.vector.tensor_scalar_add(
                        qt_sb[:, t0 : t0 + TCH], ps, bias_sb[:, 0:1]
                    )
                elif o3 == 1:
                    nc.vector.tensor_scalar_add(
                        kt_sb[:, t0 : t0 + TCH], ps, bias_sb[:, 1:2]
                    )
                else:
                    vtmp = sb_pool.tile(
                        [128, TCH], bf16, tag="vtmp", bufs=6,
                        name=f"vtmp_{tp}_{u}",
                    )
                    nc.vector.tensor_scalar_add(
                        vtmp, ps, bias_sb[:, 2:3]
                    )
                    tchk = 2 * tp + u
                    # front of the queue: the vtmp ring is shallow, so
                    # transposes must drain before more QKV chains pile
                    # up behind them in the same engine FIFOs
                    extra_fill[0:0] = [
                        ("vt", vtmp, tchk * 4 + j, j)
                        for j in range(TCH // 128)
                    ]

            # ---------------- prologue: QKV for batch 0 ----------------
            # DMA issue order tracks the first QKV chain's consumption:
            # wqkv cc0 gates the first LDWEIGHTS, then x cc-pair 0, then
            # the rest of the weights before the chain reaches cc1
            xt0 = sb_pool.tile(
                [128, NCC, TCH], bf16, tag="xt", bufs=4, name="xt_0"
            )
            # two DMA queues in parallel: sync carries wqkv + even x
            # pairs, gpsimd (idle at startup) the odd pairs + tile 1
            nc.sync.dma_start(wqkv_sb[:, 0, :], wqkv.ap()[0:128, :])
            nc.sync.dma_start(
                xt0[:, 0:2, :], xtiles.ap()[0][:, 0:2, :]
            )
            nc.gpsimd.dma_start(
                xt0[:, 2:4, :], xtiles.ap()[0][:, 2:4, :]
            )
            nc.sync.dma_start(
                wqkv_sb[:, 1:, :],
                wqkv.ap()[128:, :].rearrange("(n p) m -> p n m", p=128),
            )
            nc.gpsimd.dma_start(
                xt0[:, 6:8, :], xtiles.ap()[0][:, 6:8, :]
            )
            nc.sync.dma_start(
                xt0[:, 4:6, :], xtiles.ap()[0][:, 4:6, :]
            )
            xt1 = sb_pool.tile(
                [128, NCC, TCH], bf16, tag="xt", bufs=4, name="xt_1"
            )
            nc.gpsimd.dma_start(xt1, xtiles.ap()[1])
            xt_pairs[0] = [xt0, xt1]
            nc.sync.dma_start(
                bias_sb, bqkv.ap().rearrange("(n p) -> p n", p=128)
            )
            nc.sync.dma_start(wproj_sb, wproj.ap())
            nc.sync.dma_start(tri_sb, trimask.ap())
            # only the denominator 'ones' column needs initializing: the
            # v columns are overwritten by the transposes and the pad
            # columns are never read. The full-tile memset was 8.7us of
            # gpsimd time that blocked the startup DMAs on that queue;
            # vector (idle here) also spares gpsimd a library swap.
            nc.vector.memset(v1_sb[:, :, :, 64:65], 1.0)
            # all u=0 chains (x tile 0) before u=1 (tile 1, lands later)
            for tp in (0,):
                for u in (0, 1):
                    for o3 in range(3):
                        emit_qkv(tp, o3, u, xt_pairs[tp])
                del xt_pairs[tp]
            extra_fill.extend(
                ("qkv", tp, o3, u)
                for tp in range(1, NT // 2)
                for o3 in range(3)
                for u in (0, 1)
            )

            # -------- attention with interleaved projection + QKV --------
            def emit_proj_tile(b, qc, tb, evac_engine=0, split_evac=False):
                # PSUM from the 'bank' ring (two 1-bank halves), NOT
                # 'sgroup': sharing sgroup made the next score group's sps
                # allocation wait on this tile's DVE evacuation
                tg = b * 16 + qc * 4 + tb
                oevac = sb_pool.tile([128, C], bf16, tag="oevac", bufs=4)
                for half in range(2):
                    pp = ps_pool.tile(
                        [128, TCH], f32, tag="bank", bufs=2,
                        name=f"pp_{tg}_{half}",
                    )
                    nc.tensor.matmul(
                        pp,
                        lhsT=yt_sb[:, tg * 128 : (tg + 1) * 128],
                        rhs=wproj_sb[:, half * TCH : (half + 1) * TCH],
                        start=True,
                        stop=True,
                    )
                    oe = oevac[:, half * TCH : (half + 1) * TCH]
                    if split_evac:
                        # endgame: pipeline per-half on both free engines
                        # so the last DMAs start as early as possible
                        if half == 0:
                            nc.vector.tensor_copy(oe, pp)
                        else:
                            nc.scalar.copy(oe, pp)
                        nc.sync.dma_start(
                            out_d.ap()[
                                tg * 128 : (tg + 1) * 128,
                                half * TCH : (half + 1) * TCH,
                            ],
                            oe,
                        )
                    else:
                        if evac_engine == 0:
                            nc.vector.tensor_copy(oe, pp)
                        else:
                            # endgame/tail: ACT is done with exps there
                            nc.scalar.copy(oe, pp)
                if not split_evac:
                    # sync queue: keeps the ~0.6us DMA issues off gpsimd,
                    # whose partition_broadcasts are on the norm critical
                    # path
                    nc.sync.dma_start(
                        out_d.ap()[tg * 128 : (tg + 1) * 128, :], oevac
                    )

            def emit_proj(b, qc, evac_engine=0):
                for tb in range(4):
                    emit_proj_tile(b, qc, tb, evac_engine)

            # Software pipeline: chunk i's scores/exp run while chunk
            # i-1's AV matmuls (whose exps are long ready) fill the PE
            # between score groups, so the PE never idles on ACT.
            def emit_av(P, h, kb):
                j = kb - 4 * P["qc"]
                c0 = max(0, 128 * j)
                nc.tensor.matmul(
                    P["yps"][h][0:65, c0:],
                    lhsT=v1_sb[:, P["b"] * 16 + kb, h, 0:65],
                    rhs=P["exps"][h][:, kb, c0:],
                    start=(kb == 0),
                    stop=(kb == P["nkb"] - 1),
                )

            def emit_norm(P):
                b, qc, yps = P["b"], P["qc"], P["yps"]
                q0 = (b * 4 + qc) * TCH
                rt = sb_pool.tile(
                    [1, 2 * TCH], f32, tag="rt", bufs=2,
                    name=f"rt_{b}_{qc}",
                )
                for h in (0, 1):
                    nc.vector.tensor_copy(
                        rt[0:1, h * TCH : (h + 1) * TCH], yps[h][64:65, :]
                    )
                nc.vector.reciprocal_approx_fast(rt, rt)
                for h in (0, 1):
                    hp = h * 64
                    rb = sb_pool.tile(
                        [64, TCH], f32, tag=f"rb{h}", bufs=3,
                        name=f"rb{h}_{b}_{qc}",
                    )
                    nc.gpsimd.partition_broadcast(
                        rb, rt[0:1, h * TCH : (h + 1) * TCH], channels=64
                    )
                    nc.vector.tensor_mul(
                        yt_sb[hp : hp + 64, q0 : q0 + TCH],
                        yps[h][0:64, :],
                        rb,
                    )

            # batch-major so each batch's leftover QKV chains fill the
            # previous batches' chunks; batch 3 reversed ends the kernel
            # on a 4-block chunk (short terminal AV->norm->proj chain)
            chunks = (
                [(b, qc) for b in (0, 1, 2) for qc in range(4)]
                + [(3, qc) for qc in (3, 2, 1, 0)]
            )
            prev = None
            pending_proj = []
            for ci, (b, qc) in enumerate(chunks):
                # hard deadline (program order): this chunk's score matmuls
                # read qt[q0:q0+512] and kt[b*2048 : q0+512], so the QKV
                # chains covering that token range must be emitted before
                # the first score matmul below
                need_tp_lt = b * 2 + (qc + 2) // 2
                pending = extra_fill[:]
                extra_fill.clear()  # emit_qkv prepends fresh vt items here
                keep = []
                for it in pending:
                    if it[0] == "qkv" and it[1] < need_tp_lt:
                        emit_qkv(it[1], it[2], it[3], xt_pairs[it[1]])
                        if it[2] == 2 and it[3] == 1:
                            del xt_pairs[it[1]]
                    else:
                        keep.append(it)
                extra_fill.extend(keep)
                # soft deadline: prev chunk's AV fillers (emitted during
                # this chunk) read v1 of prev's k-range -- pull those
                # transposes to the head of this chunk's filler queue
                forced_vt = []
                if prev is not None:
                    vlim = prev["b"] * 16 + prev["nkb"]
                    keep = []
                    for it in extra_fill:
                        if it[0] == "vt" and it[2] < vlim:
                            forced_vt.append(it)
                        else:
                            keep.append(it)
                    extra_fill[:] = keep
                q0 = (b * 4 + qc) * TCH
                nkb = 4 * qc + 4
                n_g = nkb // 2
                exps = [
                    sb_pool.tile(
                        [128, 16, TCH], bf16, tag=f"exps{h}", bufs=2,
                        name=f"exps{h}_{b}_{qc}",
                    )
                    for h in (0, 1)
                ]
                yps = [
                    ps_pool.tile(
                        [128, TCH], f32, tag="yps", bufs=2,
                        name=f"yps{h}_{b}_{qc}",
                    )
                    for h in (0, 1)
                ]
                # Filler queue with per-item PE-cost weights: prev chunk's
                # AV matmuls (1), QKV chains (8), V transposes (1),
                # projection tiles (4). Front-loaded: the first score group
                # waits on the previous chunk's last EXP.
                # order: forced transposes (prev's AVs read their v1),
                # then prev's AVs -- draining them early fires the norm
                # early, giving the 2-deep yps ring slack before the NEXT
                # chunk's first AV reuses the bank -- then QKV chains and
                # projections as back-half padding
                filler = list(forced_vt)
                n_prev_av = 0
                if prev is not None:
                    P = prev
                    for kb in range(P["nkb"]):
                        for h in (0, 1):
                            filler.append(("av", P, h, kb))
                            n_prev_av += 1
                take = 4 if ci < 8 else 3
                while take and extra_fill:
                    it = extra_fill.pop(0)
                    filler.append(it)
                    if it[0] == "qkv":
                        take -= 1
                if len(pending_proj) >= 3:
                    pb, pqc = pending_proj.pop(0)
                    for tb in range(4):
                        # endgame (last two small chunks): evacuate on the
                        # scalar engine -- its exps are nearly done there,
                        # while DVE is congested with norm chains and the
                        # PE was observed starving on yt in 320-342us
                        peng = 1 if (b == 3 and qc <= 1) else 0
                        filler.append(("proj", pb, pqc, tb, peng))
                ai = 0
                avs_left = [n_prev_av]

                def cost_of(it):
                    return {"av": 1, "qkv": 8, "vt": 1, "proj": 4}[it[0]]

                def emit_item(it):
                    if it[0] == "av":
                        emit_av(it[1], it[2], it[3])
                        avs_left[0] -= 1
                        if avs_left[0] == 0:
                            # free the 2-deep yps ring early: the next
                            # chunk's accumulators reuse these banks
                            emit_norm(it[1])
                            pending_proj.append((it[1]["b"], it[1]["qc"]))
                    elif it[0] == "qkv":
                        emit_qkv(it[1], it[2], it[3], xt_pairs[it[1]])
                        if it[2] == 2 and it[3] == 1:
                            del xt_pairs[it[1]]
                    elif it[0] == "vt":
                        emit_vt1(it[1], it[2], it[3])
                    else:
                        emit_proj_tile(it[1], it[2], it[3], it[4])

                def drain(budget):
                    nonlocal ai
                    while budget > 0 and ai < len(filler):
                        it = filler[ai]
                        ai += 1
                        budget -= cost_of(it)
                        emit_item(it)

                # weighted drain schedule: g0 gets a double share (it
                # waits on the previous chunk's last EXP) but later groups
                # keep their share -- dumping everything at g0 starves them
                w = [2] + [1] * (n_g - 1)
                tw = sum(w)
                nf = sum(cost_of(it) for it in filler)
                cum = 0
                drains = []
                acc = 0
                for g in range(n_g):
                    acc += w[g]
                    nxt = nf * acc // tw
                    drains.append(nxt - cum)
                    cum = nxt
                for g in range(n_g):
                    drain(drains[g])
                    c0p = 256 if 2 * g == 4 * qc + 2 else 0
                    sps = [
                        ps_pool.tile(
                            [128, 2, TCH], f32, tag="sgroup", bufs=2,
                            name=f"sps{h}_{b}_{qc}_{g}",
                        )
                        for h in (0, 1)
                    ]
                    ih = (
                        [(0, 0), (1, 0), (0, 1), (1, 1)]
                        if g == 0
                        else [(0, 0), (0, 1), (1, 0), (1, 1)]
                    )
                    for item in ih:
                        if item is None:
                            continue
                        i, h = item
                        kb = 2 * g + i
                        j = kb - 4 * qc
                        # per-block live-column trim: block j's weights are
                        # zero below col 128j. The un-written sps columns
                        # hold stale-but-finite scores; their exps are
                        # either tri-masked or beyond the AV read range.
                        c0b = max(c0p, 128 * j) if j > 0 else c0p
                        k0 = (b * 16 + kb) * 128
                        hp = h * 64
                        nc.tensor.matmul(
                            sps[h][:, i, c0b:],
                            lhsT=kt_sb[hp : hp + 64, k0 : k0 + 128],
                            rhs=qt_sb[hp : hp + 64, q0 + c0b : q0 + TCH],
                            start=True,
                            stop=True,
                        )
                    # last group: h1's EXP first -- the next chunk's first
                    # h1 score matmuls wait on it (sps ring reuse)
                    for h in ((1, 0) if g == n_g - 1 else (0, 1)):
                        nc.scalar.activation(
                            exps[h][:, 2 * g : 2 * g + 2, c0p:],
                            sps[h][:, :, c0p:],
                            Exp,
                            scale=0.125,
                        )
                        for i in (0, 1):
                            kb = 2 * g + i
                            j = kb - 4 * qc
                            if j >= 0:
                                c0 = 128 * j
                                nc.vector.tensor_mul(
                                    exps[h][:, kb, c0 : c0 + 128],
                                    exps[h][:, kb, c0 : c0 + 128],
                                    tri_sb,
                                )
                drain(10 ** 9)
                prev = {
                    "b": b, "qc": qc, "exps": exps, "yps": yps, "nkb": nkb,
                }
            # tail: interleave the final chunk's AV with the ready
            # projections of older chunks, then norm + the last projections
            tail_av = [(h, kb) for kb in range(prev["nkb"]) for h in (0, 1)]
            tail_proj = []
            for pb, pqc in pending_proj[:-1]:
                for tb in range(4):
                    tail_proj.append((pb, pqc, tb))
            ti = pi = 0
            while ti < len(tail_av) or pi < len(tail_proj):
                for _ in range(3):
                    if ti < len(tail_av):
                        emit_av(prev, *tail_av[ti])
                        ti += 1
                if pi < len(tail_proj):
                    emit_proj_tile(*tail_proj[pi], evac_engine=pi % 2)
                    pi += 1
            emit_norm(prev)
            last = pending_proj[-1] if pending_proj else None
            pending_proj.append((prev["b"], prev["qc"]))
            if last is not None:
                for tb in range(4):
                    emit_proj_tile(last[0], last[1], tb, split_evac=True)
            for tb in range(4):
                emit_proj_tile(prev["b"], prev["qc"], tb, split_evac=True)
    nc.compile()
    return nc


class Runner:
    """Builds the Bass program once and keeps a reusable jitted executor."""

    def __init__(self):
        self.nc = _build_nc()
        self._jit = None
        self._meta = None

    def _build_jit(self):
        import jax
        import numpy as np
        from jax.sharding import Mesh, PartitionSpec
        from jax.experimental.shard_map import shard_map
        import concourse.mybir as mybir
        from concourse import bass2jax

        nc = self.nc
        bass2jax.install_neuronx_cc_hook()

        partition_name = (
            nc.partition_id_tensor.name if nc.partition_id_tensor else None
        )
        in_names, out_names, out_avals = [], [], []
        for alloc in nc.m.functions[0].allocations:
            if not isinstance(alloc, mybir.MemoryLocationSet):
                continue
            name = alloc.memorylocations[0].name
            if alloc.kind == "ExternalInput":
                if name != partition_name:
                    in_names.append(name)
            elif alloc.kind == "ExternalOutput":
                out_names.append(name)
                out_avals.append(
                    jax.core.ShapedArray(
                        tuple(alloc.tensor_shape), mybir.dt.np(alloc.dtype)
                    )
                )
        n_params = len(in_names)
        n_outs = len(out_avals)
        all_in = list(in_names) + list(out_names)
        if partition_name is not None:
            all_in.append(partition_name)

        def _body(*args):
            operands = list(args)
            if partition_name is not None:
                operands.append(bass2jax.partition_id_tensor())
            outs = bass2jax._bass_exec_p.bind(
                *operands,
                out_avals=tuple(out_avals),
                in_names=tuple(all_in),
                out_names=tuple(out_names),
                lowering_input_output_aliases=(),
                sim_require_finite=True,
                sim_require_nnan=True,
                nc=nc,
            )
            return tuple(outs)

        devices = jax.devices()[:NCORES]
        mesh = Mesh(np.asarray(devices), ("core",))
        donate = tuple(range(n_params, n_params + n_outs))
        sharded = jax.jit(
            shard_map(
                _body,
                mesh=mesh,
                in_specs=(PartitionSpec("core"),) * (n_params + n_outs),
                out_specs=(PartitionSpec("core"),) * n_outs,
                check_rep=False,
            ),
            donate_argnums=donate,
            keep_unused=True,
        )
        self._jit = sharded
        self._meta = (in_names, out_names, out_avals)

    def build_timer(self, in_maps, iters):
        """Returns a zero-transfer callable running `iters` chained kernel
        executions on device; inputs are staged on device once."""
        import jax
        import jax.numpy as jnp
        import numpy as np
        from jax.sharding import Mesh, PartitionSpec, NamedSharding
        from jax.experimental.shard_map import shard_map
        import concourse.mybir as mybir
        from concourse import bass2jax

        if self._jit is None:
            self._build_jit()
        nc = self.nc
        in_names, out_names, out_avals = self._meta
        partition_name = (
            nc.partition_id_tensor.name if nc.partition_id_tensor else None
        )
        all_in = list(in_names) + list(out_names)
        if partition_name is not None:
            all_in.append(partition_name)

        n_params = len(in_names)

        def _body(*args):
            ins = list(args[:n_params])
            zeros = list(args[n_params:])
            outs = None
            for _ in range(iters):
                operands = list(ins) + list(zeros)
                if partition_name is not None:
                    operands.append(bass2jax.partition_id_tensor())
                outs = bass2jax._bass_exec_p.bind(
                    *operands,
                    out_avals=tuple(out_avals),
                    in_names=tuple(all_in),
                    out_names=tuple(out_names),
                    lowering_input_output_aliases=(),
                    sim_require_finite=True,
                    sim_require_nnan=True,
                    nc=nc,
                )
            return tuple(outs)

        devices = jax.devices()[:NCORES]
        mesh = Mesh(np.asarray(devices), ("core",))
        spec = NamedSharding(mesh, PartitionSpec("core"))
        fn = jax.jit(
            shard_map(
                _body,
                mesh=mesh,
                in_specs=(PartitionSpec("core"),)
                * (len(in_names) + len(out_names)),
                out_specs=(PartitionSpec("core"),) * len(out_names),
                check_rep=False,
            ),
            keep_unused=True,
        )
        concat_in = [
            jax.device_put(
                np.concatenate([np.asarray(m[name]) for m in in_maps], axis=0),
                spec,
            )
            for name in in_names
        ]
        concat_in += [
            jax.device_put(
                np.zeros((NCORES * a.shape[0], *a.shape[1:]), a.dtype), spec
            )
            for a in out_avals
        ]
        for a in concat_in:
            a.block_until_ready()

        def run():
            outs = fn(*concat_in)
            jax.block_until_ready(outs)
            return outs

        return run

    def execute(self, in_maps):
        """in_maps: list of 8 dicts name->np array. Returns list of out dicts."""
        import numpy as np

        if self._jit is None:
            self._build_jit()
        in_names, out_names, out_avals = self._meta
        concat_in = [
            np.concatenate([np.asarray(m[name]) for m in in_maps], axis=0)
            for name in in_names
        ]
        concat_zeros = [
            np.zeros((NCORES * a.shape[0], *a.shape[1:]), a.dtype)
            for a in out_avals
        ]
        out_arrs = self._jit(*concat_in, *concat_zeros)
        return [
            {
                name: np.asarray(out_arrs[i]).reshape(
                    NCORES, *out_avals[i].shape
                )[c]
                for i, name in enumerate(out_names)
            }
            for c in range(NCORES)
        ]


def make_in_maps(x, W_attn, b_attn, W_proj, b_proj):
    bf16 = ml_dtypes.bfloat16
    xTb = x.reshape(BT, C).T.astype(bf16)  # [C, BT]
    # tiled layout: [tch, p, cc, t] = xT[cc*128+p, tch*512+t], contiguous
    xtiles = np.ascontiguousarray(
        xTb.reshape(NCC, 128, NT, TCH).transpose(2, 1, 0, 3)
    )
    tri = np.tril(np.ones((128, 128), np.float32)).T.astype(bf16)
    # trimask[p, c] = 1 if p <= c  (k index on partitions, q on cols)
    in_maps = []
    for c in range(NCORES):
        h0 = 2 * c
        cols = np.r_[h0 * HD : (h0 + 2) * HD]
        wq = W_attn[:, cols]
        wk = W_attn[:, C + cols]
        wv = W_attn[:, 2 * C + cols]
        wqkv = np.concatenate([wq, wk, wv], axis=1).astype(bf16)
        bqkv = np.concatenate(
            [b_attn[cols], b_attn[C + cols], b_attn[2 * C + cols]]
        ).astype(np.float32)
        wproj = np.ascontiguousarray(W_proj[cols, :]).astype(bf16)
        in_maps.append(
            {
                "xtiles": xtiles,
                "wqkv": np.ascontiguousarray(wqkv),
                "bqkv": bqkv,
                "wproj": wproj,
                "trimask": np.ascontiguousarray(tri),
            }
        )
    return in_maps


def get_runner():
    global _RUNNER
    if _RUNNER is None:
        _RUNNER = Runner()
    return _RUNNER


def kernel(x, W_attn, b_attn, W_proj, b_proj):
    x = np.asarray(x, dtype=np.float32)
    W_attn = np.asarray(W_attn, dtype=np.float32)
    b_attn = np.asarray(b_attn, dtype=np.float32)
    W_proj = np.asarray(W_proj, dtype=np.float32)
    b_proj = np.asarray(b_proj, dtype=np.float32)
    runner = get_runner()
    in_maps = make_in_maps(x, W_attn, b_attn, W_proj, b_proj)
    results = runner.execute(in_maps)
    total = np.zeros((BT, C), np.float32)
    for r in results:
        total += np.asarray(r["out"], dtype=np.float32)
    total += b_proj[None, :]
    return total.reshape(B, T, C)


# revision 41
# speedup vs baseline: 1.0018x; 1.0018x over previous
"""Causal self-attention (B=4, T=2048, C=1024, H=16) on 8 trn2 NeuronCores.

Sharding: tensor-parallel over heads. Core c owns heads (2c, 2c+1).
Each core computes QKV projection for its 2 heads (full x), causal
attention for its (4 batches x 2 heads), and a partial output projection
with its 128 rows of W_proj. Host sums the 8 bf16 partial outputs
(+ b_proj) in fp32.

Phase-merged schedule: only the first two t-pair QKV chains (batch 0's
tokens) run as a prologue; the remaining six (tps 2-7) and their V
transposes are folded into the attention loop's filler queue. The PE
then always has dense matmul work between score groups (instead of
idling on ACT latency), which both raises overlap and keeps the HAM
clock gate at K=8/8 (half-clock oscillation cost ~50us in the phased
schedule). Deadlines force any remaining QKV/transpose items before the
first chunk of the batch that needs them.

Device-side layout choices:
  - x is transposed host-side; loaded as [c_in, t] tiles (bf16).
  - Q,K produced transposed: [c_out(=2*64), t]; the two heads live on
    partition halves 0:64 / 64:128, so the two heads' score matmuls
    row-pack (concurrent row-tiled MMs) in the PE array.
  - scores S_T [k, q] per 128-k-block, diagonal blocks truncated to
    their live column range; exp on ACT (scale folded); causal triangle
    via bf16 mask multiplies on DVE.
  - softmax denominators ride the AV matmul as a 'ones' column of V1;
    reciprocal on DVE; broadcast over partitions via GPSIMD
    partition_broadcast (no PE involvement).
  - AV accumulates y_T unnormalized; normalization fused into the
    PSUM->SBUF evacuation multiply, emitted as soon as the previous
    chunk's AV fillers drain (the 2-deep always-'yps' PSUM ring needs
    the early release).
  - PSUM tags: sps pairs on 'sgroup' (4 banks), AV accumulators always
    on 'yps' (2 banks), QKV chains + PE transposes on 'bank' (2 banks).
  - projection interleaved chunk-by-chunk into the attention loop;
    chunk order is batch-major with batch 3 reversed (ends on a 4-block
    chunk), and tail projections alternate scalar/vector evacuation.
"""

import sys

sys.path.insert(0, "/opt/trn_rl_repo")

import numpy as np
import ml_dtypes

B, T, C, H = 4, 2048, 1024, 16
HD = C // H  # 64
BT = B * T  # 8192
NCORES = 8
TCH = 512  # t-chunk
NT = BT // TCH  # 16
NCC = C // 128  # 8 c_in chunks
KB = 128  # k block

_RUNNER = None


def _build_nc():
    import concourse.bacc as bacc
    import concourse.mybir as mybir
    import concourse.tile as tile
    from concourse.masks import make_identity

    f32 = mybir.dt.float32
    bf16 = mybir.dt.bfloat16
    Exp = mybir.ActivationFunctionType.Exp

    nc = bacc.Bacc(None, target_bir_lowering=False, debug=False)

    xtiles = nc.dram_tensor("xtiles", [NT, 128, NCC, TCH], bf16, kind="ExternalInput")
    wqkv = nc.dram_tensor("wqkv", [C, 384], bf16, kind="ExternalInput")
    bqkv = nc.dram_tensor("bqkv", [384], f32, kind="ExternalInput")
    wproj = nc.dram_tensor("wproj", [128, C], bf16, kind="ExternalInput")
    trimask = nc.dram_tensor("trimask", [128, 128], bf16, kind="ExternalInput")
    out_d = nc.dram_tensor("out", [BT, C], bf16, kind="ExternalOutput")

    with tile.TileContext(nc) as tc:
        with (
            tc.tile_pool(name="const", bufs=1) as const_pool,
            tc.tile_pool(name="big", bufs=1) as big_pool,
            tc.tile_pool(name="sb", bufs=2) as sb_pool,
            tc.tile_pool(name="ps", bufs=1, space="PSUM") as ps_pool,
        ):
            # --- constants ---
            wqkv_sb = const_pool.tile([128, NCC, 384], bf16)
            bias_sb = const_pool.tile([128, 3], f32)
            ident_sb = const_pool.tile([128, 128], bf16)
            make_identity(nc, ident_sb)
            wproj_sb = const_pool.tile([128, C], bf16)
            tri_sb = const_pool.tile([128, 128], bf16)

            # --- persistent activations ---
            qt_sb = big_pool.tile([128, BT], bf16)  # Q_T [2*64, t]
            kt_sb = big_pool.tile([128, BT], bf16)  # K_T
            # V1 per (t-block, head): cols 0:64 V_h, col 64 ones, pad to 80
            v1_sb = big_pool.tile([128, BT // 128, 2, 80], bf16)
            yt_sb = big_pool.tile([128, BT], bf16)  # y_T [c, t] normalized
            # (v1 memset is emitted after the prologue DMA issues so the
            # gpsimd queue's startup DMAs go out first)

            # ---------------- QKV building blocks ----------------
            def emit_vt1(vtmp, tb, j):
                vtp = ps_pool.tile(
                    [128, 128], bf16, tag="bank", bufs=2,
                    name=f"vtp_{tb}",
                )
                nc.tensor.transpose(
                    vtp, vtmp[:, j * 128 : (j + 1) * 128], ident_sb
                )
                # split rows (=heads) into V1 [128,2,64]
                dst = v1_sb[:, tb, :, 0:64]
                src = vtp.rearrange("p (h d) -> p h d", h=2)
                if j % 2 == 0:
                    nc.scalar.copy(dst, src)
                else:
                    nc.vector.tensor_copy(dst, src)

            def fetch_xt(tch, split=False):
                xt = sb_pool.tile(
                    [128, NCC, TCH], bf16, tag="xt", bufs=4,
                    name=f"xt_{tch}",
                )
                if split:
                    # first tile lands cc-pair by cc-pair so the first
                    # QKV chain starts earlier
                    for ccp in range(NCC // 2):
                        nc.sync.dma_start(
                            xt[:, 2 * ccp : 2 * ccp + 2, :],
                            xtiles.ap()[tch][:, 2 * ccp : 2 * ccp + 2, :],
                        )
                else:
                    nc.sync.dma_start(xt, xtiles.ap()[tch])
                return xt

            xt_pairs = {}  # tp -> [xt, xt]
            extra_fill = []  # deque of ("qkv", tp, o3) / ("vt", vtmp, tb, j)

            def emit_qkv(tp, o3, u, xts):
                # one output block (q, k or v cols) for ONE t-chunk: a
                # ~1.7us half-chain, fine-grained enough for the drain
                # scheduler to place accurately between score groups
                if o3 == 0 and u == 0 and tp + 1 < NT // 2 \
                        and tp + 1 not in xt_pairs:
                    xt_pairs[tp + 1] = [
                        fetch_xt(2 * tp + 2), fetch_xt(2 * tp + 3)
                    ]
                ps = ps_pool.tile(
                    [128, TCH], f32, tag="bank", bufs=2,
                    name=f"qkv_{tp}_{o3}_{u}",
                )
                for cc in range(NCC):
                    nc.tensor.matmul(
                        ps,
                        lhsT=wqkv_sb[:, cc, o3 * 128 : (o3 + 1) * 128],
                        rhs=xts[u][:, cc, :],
                        start=(cc == 0),
                        stop=(cc == NCC - 1),
                    )
                t0 = (2 * tp + u) * TCH
                if o3 == 0:
                    nc.vector.tensor_scalar_add(
                        qt_sb[:, t0 : t0 + TCH], ps, bias_sb[:, 0:1]
                    )
                elif o3 == 1:
                    nc.vector.tensor_scalar_add(
                        kt_sb[:, t0 : t0 + TCH], ps, bias_sb[:, 1:2]
                    )
                else:
                    vtmp = sb_pool.tile(
                        [128, TCH], bf16, tag="vtmp", bufs=6,
                        name=f"vtmp_{tp}_{u}",
                    )
                    nc.vector.tensor_scalar_add(
                        vtmp, ps, bias_sb[:, 2:3]
                    )
                    tchk = 2 * tp + u
                    # front of the queue: the vtmp ring is shallow, so
                    # transposes must drain before more QKV chains pile
                    # up behind them in the same engine FIFOs
                    extra_fill[0:0] = [
                        ("vt", vtmp, tchk * 4 + j, j)
                        for j in range(TCH // 128)
                    ]

            # ---------------- prologue: QKV for batch 0 ----------------
            # DMA issue order tracks the first QKV chain's consumption:
            # wqkv cc0 gates the first LDWEIGHTS, then x cc-pair 0, then
            # the rest of the weights before the chain reaches cc1
            xt0 = sb_pool.tile(
                [128, NCC, TCH], bf16, tag="xt", bufs=4, name="xt_0"
            )
            # two DMA queues in parallel: sync carries wqkv + even x
            # pairs, gpsimd (idle at startup) the odd pairs + tile 1
            nc.sync.dma_start(wqkv_sb[:, 0, :], wqkv.ap()[0:128, :])
            nc.sync.dma_start(
                xt0[:, 0:2, :], xtiles.ap()[0][:, 0:2, :]
            )
            nc.gpsimd.dma_start(
                xt0[:, 2:4, :], xtiles.ap()[0][:, 2:4, :]
            )
            nc.sync.dma_start(
                wqkv_sb[:, 1:, :],
                wqkv.ap()[128:, :].rearrange("(n p) m -> p n m", p=128),
            )
            nc.gpsimd.dma_start(
                xt0[:, 6:8, :], xtiles.ap()[0][:, 6:8, :]
            )
            nc.sync.dma_start(
                xt0[:, 4:6, :], xtiles.ap()[0][:, 4:6, :]
            )
            xt1 = sb_pool.tile(
                [128, NCC, TCH], bf16, tag="xt", bufs=4, name="xt_1"
            )
            nc.gpsimd.dma_start(xt1, xtiles.ap()[1])
            xt_pairs[0] = [xt0, xt1]
            nc.sync.dma_start(
                bias_sb, bqkv.ap().rearrange("(n p) -> p n", p=128)
            )
            nc.sync.dma_start(wproj_sb, wproj.ap())
            nc.sync.dma_start(tri_sb, trimask.ap())
            # only the denominator 'ones' column needs initializing: the
            # v columns are overwritten by the transposes and the pad
            # columns are never read. The full-tile memset was 8.7us of
            # gpsimd time that blocked the startup DMAs on that queue.
            nc.gpsimd.memset(v1_sb[:, :, :, 64:65], 1.0)
            # all u=0 chains (x tile 0) before u=1 (tile 1, lands later)
            for tp in (0,):
                for u in (0, 1):
                    for o3 in range(3):
                        emit_qkv(tp, o3, u, xt_pairs[tp])
                del xt_pairs[tp]
            extra_fill.extend(
                ("qkv", tp, o3, u)
                for tp in range(1, NT // 2)
                for o3 in range(3)
                for u in (0, 1)
            )

            # -------- attention with interleaved projection + QKV --------
            def emit_proj_tile(b, qc, tb, evac_engine=0, split_evac=False):
                # PSUM from the 'bank' ring (two 1-bank halves), NOT
                # 'sgroup': sharing sgroup made the next score group's sps
                # allocation wait on this tile's DVE evacuation
                tg = b * 16 + qc * 4 + tb
                oevac = sb_pool.tile([128, C], bf16, tag="oevac", bufs=4)
                for half in range(2):
                    pp = ps_pool.tile(
                        [128, TCH], f32, tag="bank", bufs=2,
                        name=f"pp_{tg}_{half}",
                    )
                    nc.tensor.matmul(
                        pp,
                        lhsT=yt_sb[:, tg * 128 : (tg + 1) * 128],
                        rhs=wproj_sb[:, half * TCH : (half + 1) * TCH],
                        start=True,
                        stop=True,
                    )
                    oe = oevac[:, half * TCH : (half + 1) * TCH]
                    if split_evac:
                        # endgame: pipeline per-half on both free engines
                        # so the last DMAs start as early as possible
                        if half == 0:
                            nc.vector.tensor_copy(oe, pp)
                        else:
                            nc.scalar.copy(oe, pp)
                        nc.sync.dma_start(
                            out_d.ap()[
                                tg * 128 : (tg + 1) * 128,
                                half * TCH : (half + 1) * TCH,
                            ],
                            oe,
                        )
                    else:
                        if evac_engine == 0:
                            nc.vector.tensor_copy(oe, pp)
                        else:
                            # endgame/tail: ACT is done with exps there
                            nc.scalar.copy(oe, pp)
                if not split_evac:
                    # sync queue: keeps the ~0.6us DMA issues off gpsimd,
                    # whose partition_broadcasts are on the norm critical
                    # path
                    nc.sync.dma_start(
                        out_d.ap()[tg * 128 : (tg + 1) * 128, :], oevac
                    )

            def emit_proj(b, qc, evac_engine=0):
                for tb in range(4):
                    emit_proj_tile(b, qc, tb, evac_engine)

            # Software pipeline: chunk i's scores/exp run while chunk
            # i-1's AV matmuls (whose exps are long ready) fill the PE
            # between score groups, so the PE never idles on ACT.
            def emit_av(P, h, kb):
                j = kb - 4 * P["qc"]
                c0 = max(0, 128 * j)
                nc.tensor.matmul(
                    P["yps"][h][0:65, c0:],
                    lhsT=v1_sb[:, P["b"] * 16 + kb, h, 0:65],
                    rhs=P["exps"][h][:, kb, c0:],
                    start=(kb == 0),
                    stop=(kb == P["nkb"] - 1),
                )

            def emit_norm(P):
                b, qc, yps = P["b"], P["qc"], P["yps"]
                q0 = (b * 4 + qc) * TCH
                rt = sb_pool.tile(
                    [1, 2 * TCH], f32, tag="rt", bufs=2,
                    name=f"rt_{b}_{qc}",
                )
                for h in (0, 1):
                    nc.vector.tensor_copy(
                        rt[0:1, h * TCH : (h + 1) * TCH], yps[h][64:65, :]
                    )
                nc.vector.reciprocal_approx_fast(rt, rt)
                for h in (0, 1):
                    hp = h * 64
                    rb = sb_pool.tile(
                        [64, TCH], f32, tag=f"rb{h}", bufs=3,
                        name=f"rb{h}_{b}_{qc}",
                    )
                    nc.gpsimd.partition_broadcast(
                        rb, rt[0:1, h * TCH : (h + 1) * TCH], channels=64
                    )
                    nc.vector.tensor_mul(
                        yt_sb[hp : hp + 64, q0 : q0 + TCH],
                        yps[h][0:64, :],
                        rb,
                    )

            # batch-major so each batch's leftover QKV chains fill the
            # previous batches' chunks; batch 3 reversed ends the kernel
            # on a 4-block chunk (short terminal AV->norm->proj chain)
            chunks = (
                [(b, qc) for b in (0, 1, 2) for qc in range(4)]
                + [(3, qc) for qc in (3, 2, 1, 0)]
            )
            prev = None
            pending_proj = []
            for ci, (b, qc) in enumerate(chunks):
                # hard deadline (program order): this chunk's score matmuls
                # read qt[q0:q0+512] and kt[b*2048 : q0+512], so the QKV
                # chains covering that token range must be emitted before
                # the first score matmul below
                need_tp_lt = b * 2 + (qc + 2) // 2
                pending = extra_fill[:]
                extra_fill.clear()  # emit_qkv prepends fresh vt items here
                keep = []
                for it in pending:
                    if it[0] == "qkv" and it[1] < need_tp_lt:
                        emit_qkv(it[1], it[2], it[3], xt_pairs[it[1]])
                        if it[2] == 2 and it[3] == 1:
                            del xt_pairs[it[1]]
                    else:
                        keep.append(it)
                extra_fill.extend(keep)
                # soft deadline: prev chunk's AV fillers (emitted during
                # this chunk) read v1 of prev's k-range -- pull those
                # transposes to the head of this chunk's filler queue
                forced_vt = []
                if prev is not None:
                    vlim = prev["b"] * 16 + prev["nkb"]
                    keep = []
                    for it in extra_fill:
                        if it[0] == "vt" and it[2] < vlim:
                            forced_vt.append(it)
                        else:
                            keep.append(it)
                    extra_fill[:] = keep
                q0 = (b * 4 + qc) * TCH
                nkb = 4 * qc + 4
                n_g = nkb // 2
                exps = [
                    sb_pool.tile(
                        [128, 16, TCH], bf16, tag=f"exps{h}", bufs=2,
                        name=f"exps{h}_{b}_{qc}",
                    )
                    for h in (0, 1)
                ]
                yps = [
                    ps_pool.tile(
                        [128, TCH], f32, tag="yps", bufs=2,
                        name=f"yps{h}_{b}_{qc}",
                    )
                    for h in (0, 1)
                ]
                # Filler queue with per-item PE-cost weights: prev chunk's
                # AV matmuls (1), QKV chains (8), V transposes (1),
                # projection tiles (4). Front-loaded: the first score group
                # waits on the previous chunk's last EXP.
                # order: forced transposes (prev's AVs read their v1),
                # then prev's AVs -- draining them early fires the norm
                # early, giving the 2-deep yps ring slack before the NEXT
                # chunk's first AV reuses the bank -- then QKV chains and
                # projections as back-half padding
                filler = list(forced_vt)
                n_prev_av = 0
                if prev is not None:
                    P = prev
                    for kb in range(P["nkb"]):
                        for h in (0, 1):
                            filler.append(("av", P, h, kb))
                            n_prev_av += 1
                take = 4 if ci < 8 else 3
                while take and extra_fill:
                    it = extra_fill.pop(0)
                    filler.append(it)
                    if it[0] == "qkv":
                        take -= 1
                if len(pending_proj) >= 3:
                    pb, pqc = pending_proj.pop(0)
                    for tb in range(4):
                        # endgame (last two small chunks): evacuate on the
                        # scalar engine -- its exps are nearly done there,
                        # while DVE is congested with norm chains and the
                        # PE was observed starving on yt in 320-342us
                        peng = 1 if (b == 3 and qc <= 1) else 0
                        filler.append(("proj", pb, pqc, tb, peng))
                ai = 0
                avs_left = [n_prev_av]

                def cost_of(it):
                    return {"av": 1, "qkv": 8, "vt": 1, "proj": 4}[it[0]]

                def emit_item(it):
                    if it[0] == "av":
                        emit_av(it[1], it[2], it[3])
                        avs_left[0] -= 1
                        if avs_left[0] == 0:
                            # free the 2-deep yps ring early: the next
                            # chunk's accumulators reuse these banks
                            emit_norm(it[1])
                            pending_proj.append((it[1]["b"], it[1]["qc"]))
                    elif it[0] == "qkv":
                        emit_qkv(it[1], it[2], it[3], xt_pairs[it[1]])
                        if it[2] == 2 and it[3] == 1:
                            del xt_pairs[it[1]]
                    elif it[0] == "vt":
                        emit_vt1(it[1], it[2], it[3])
                    else:
                        emit_proj_tile(it[1], it[2], it[3], it[4])

                def drain(budget):
                    nonlocal ai
                    while budget > 0 and ai < len(filler):
                        it = filler[ai]
                        ai += 1
                        budget -= cost_of(it)
                        emit_item(it)

                # weighted drain schedule: g0 gets a double share (it
                # waits on the previous chunk's last EXP) but later groups
                # keep their share -- dumping everything at g0 starves them
                w = [2] + [1] * (n_g - 1)
                tw = sum(w)
                nf = sum(cost_of(it) for it in filler)
                cum = 0
                drains = []
                acc = 0
                for g in range(n_g):
                    acc += w[g]
                    nxt = nf * acc // tw
                    drains.append(nxt - cum)
                    cum = nxt
                for g in range(n_g):
                    drain(drains[g])
                    c0p = 256 if 2 * g == 4 * qc + 2 else 0
                    sps = [
                        ps_pool.tile(
                            [128, 2, TCH], f32, tag="sgroup", bufs=2,
                            name=f"sps{h}_{b}_{qc}_{g}",
                        )
                        for h in (0, 1)
                    ]
                    ih = (
                        [(0, 0), (1, 0), (0, 1), (1, 1)]
                        if g == 0
                        else [(0, 0), (0, 1), (1, 0), (1, 1)]
                    )
                    for item in ih:
                        if item is None:
                            continue
                        i, h = item
                        kb = 2 * g + i
                        j = kb - 4 * qc
                        # per-block live-column trim: block j's weights are
                        # zero below col 128j. The un-written sps columns
                        # hold stale-but-finite scores; their exps are
                        # either tri-masked or beyond the AV read range.
                        c0b = max(c0p, 128 * j) if j > 0 else c0p
                        k0 = (b * 16 + kb) * 128
                        hp = h * 64
                        nc.tensor.matmul(
                            sps[h][:, i, c0b:],
                            lhsT=kt_sb[hp : hp + 64, k0 : k0 + 128],
                            rhs=qt_sb[hp : hp + 64, q0 + c0b : q0 + TCH],
                            start=True,
                            stop=True,
                        )
                    # last group: h1's EXP first -- the next chunk's first
                    # h1 score matmuls wait on it (sps ring reuse)
                    for h in ((1, 0) if g == n_g - 1 else (0, 1)):
                        nc.scalar.activation(
                            exps[h][:, 2 * g : 2 * g + 2, c0p:],
                            sps[h][:, :, c0p:],
                            Exp,
                            scale=0.125,
                        )
                        for i in (0, 1):
                            kb = 2 * g + i
                            j = kb - 4 * qc
                            if j >= 0:
                                c0 = 128 * j
                                nc.vector.tensor_mul(
                                    exps[h][:, kb, c0 : c0 + 128],
                                    exps[h][:, kb, c0 : c0 + 128],
                                    tri_sb,
                                )
                drain(10 ** 9)
                prev = {
                    "b": b, "qc": qc, "exps": exps, "yps": yps, "nkb": nkb,
                }
            # tail: interleave the final chunk's AV with the ready
            # projections of older chunks, then norm + the last projections
            tail_av = [(h, kb) for kb in range(prev["nkb"]) for h in (0, 1)]
            tail_proj = []
            for pb, pqc in pending_proj[:-1]:
                for tb in range(4):
                    tail_proj.append((pb, pqc, tb))
            ti = pi = 0
            while ti < len(tail_av) or pi < len(tail_proj):
                for _ in range(3):
                    if ti < len(tail_av):
                        emit_av(prev, *tail_av[ti])
                        ti += 1
                if pi < len(tail_proj):
                    emit_proj_tile(*tail_proj[pi], evac_engine=pi % 2)
                    pi += 1
            emit_norm(prev)
            last = pending_proj[-1] if pending_proj else None
            pending_proj.append((prev["b"], prev["qc"]))
            if last is not None:
                for tb in range(4):
                    emit_proj_tile(last[0], last[1], tb, split_evac=True)
            for tb in range(4):
                emit_proj_tile(prev["b"], prev["qc"], tb, split_evac=True)
    nc.compile()
    return nc


class Runner:
    """Builds the Bass program once and keeps a reusable jitted executor."""

    def __init__(self):
        self.nc = _build_nc()
        self._jit = None
        self._meta = None

    def _build_jit(self):
        import jax
        import numpy as np
        from jax.sharding import Mesh, PartitionSpec
        from jax.experimental.shard_map import shard_map
        import concourse.mybir as mybir
        from concourse import bass2jax

        nc = self.nc
        bass2jax.install_neuronx_cc_hook()

        partition_name = (
            nc.partition_id_tensor.name if nc.partition_id_tensor else None
        )
        in_names, out_names, out_avals = [], [], []
        for alloc in nc.m.functions[0].allocations:
            if not isinstance(alloc, mybir.MemoryLocationSet):
                continue
            name = alloc.memorylocations[0].name
            if alloc.kind == "ExternalInput":
                if name != partition_name:
                    in_names.append(name)
            elif alloc.kind == "ExternalOutput":
                out_names.append(name)
                out_avals.append(
                    jax.core.ShapedArray(
                        tuple(alloc.tensor_shape), mybir.dt.np(alloc.dtype)
                    )
                )
        n_params = len(in_names)
        n_outs = len(out_avals)
        all_in = list(in_names) + list(out_names)
        if partition_name is not None:
            all_in.append(partition_name)

        def _body(*args):
            operands = list(args)
            if partition_name is not None:
                operands.append(bass2jax.partition_id_tensor())
            outs = bass2jax._bass_exec_p.bind(
                *operands,
                out_avals=tuple(out_avals),
                in_names=tuple(all_in),
                out_names=tuple(out_names),
                lowering_input_output_aliases=(),
                sim_require_finite=True,
                sim_require_nnan=True,
                nc=nc,
            )
            return tuple(outs)

        devices = jax.devices()[:NCORES]
        mesh = Mesh(np.asarray(devices), ("core",))
        donate = tuple(range(n_params, n_params + n_outs))
        sharded = jax.jit(
            shard_map(
                _body,
                mesh=mesh,
                in_specs=(PartitionSpec("core"),) * (n_params + n_outs),
                out_specs=(PartitionSpec("core"),) * n_outs,
                check_rep=False,
            ),
            donate_argnums=donate,
            keep_unused=True,
        )
        self._jit = sharded
        self._meta = (in_names, out_names, out_avals)

    def build_timer(self, in_maps, iters):
        """Returns a zero-transfer callable running `iters` chained kernel
        executions on device; inputs are staged on device once."""
        import jax
        import jax.numpy as jnp
        import numpy as np
        from jax.sharding import Mesh, PartitionSpec, NamedSharding
        from jax.experimental.shard_map import shard_map
        import concourse.mybir as mybir
        from concourse import bass2jax

        if self._jit is None:
            self._build_jit()
        nc = self.nc
        in_names, out_names, out_avals = self._meta
        partition_name = (
            nc.partition_id_tensor.name if nc.partition_id_tensor else None
        )
        all_in = list(in_names) + list(out_names)
        if partition_name is not None:
            all_in.append(partition_name)

        n_params = len(in_names)

        def _body(*args):
            ins = list(args[:n_params])
            zeros = list(args[n_params:])
            outs = None
            for _ in range(iters):
                operands = list(ins) + list(zeros)
                if partition_name is not None:
                    operands.append(bass2jax.partition_id_tensor())
                outs = bass2jax._bass_exec_p.bind(
                    *operands,
                    out_avals=tuple(out_avals),
                    in_names=tuple(all_in),
                    out_names=tuple(out_names),
                    lowering_input_output_aliases=(),
                    sim_require_finite=True,
                    sim_require_nnan=True,
                    nc=nc,
                )
            return tuple(outs)

        devices = jax.devices()[:NCORES]
        mesh = Mesh(np.asarray(devices), ("core",))
        spec = NamedSharding(mesh, PartitionSpec("core"))
        fn = jax.jit(
            shard_map(
                _body,
                mesh=mesh,
                in_specs=(PartitionSpec("core"),)
                * (len(in_names) + len(out_names)),
                out_specs=(PartitionSpec("core"),) * len(out_names),
                check_rep=False,
            ),
            keep_unused=True,
        )
        concat_in = [
            jax.device_put(
                np.concatenate([np.asarray(m[name]) for m in in_maps], axis=0),
                spec,
            )
            for name in in_names
        ]
        concat_in += [
            jax.device_put(
                np.zeros((NCORES * a.shape[0], *a.shape[1:]), a.dtype), spec
            )
            for a in out_avals
        ]
        for a in concat_in:
            a.block_until_ready()

        def run():
            outs = fn(*concat_in)
            jax.block_until_ready(outs)
            return outs

        return run

    def execute(self, in_maps):
        """in_maps: list of 8 dicts name->np array. Returns list of out dicts."""
        import numpy as np

        if self._jit is None:
            self._build_jit()
        in_names, out_names, out_avals = self._meta
        concat_in = [
            np.concatenate([np.asarray(m[name]) for m in in_maps], axis=0)
            for name in in_names
        ]
        concat_zeros = [
            np.zeros((NCORES * a.shape[0], *a.shape[1:]), a.dtype)
            for a in out_avals
        ]
        out_arrs = self._jit(*concat_in, *concat_zeros)
        return [
            {
                name: np.asarray(out_arrs[i]).reshape(
                    NCORES, *out_avals[i].shape
                )[c]
                for i, name in enumerate(out_names)
            }
            for c in range(NCORES)
        ]


def make_in_maps(x, W_attn, b_attn, W_proj, b_proj):
    bf16 = ml_dtypes.bfloat16
    xTb = x.reshape(BT, C).T.astype(bf16)  # [C, BT]
    # tiled layout: [tch, p, cc, t] = xT[cc*128+p, tch*512+t], contiguous
    xtiles = np.ascontiguousarray(
        xTb.reshape(NCC, 128, NT, TCH).transpose(2, 1, 0, 3)
    )
    tri = np.tril(np.ones((128, 128), np.float32)).T.astype(bf16)
    # trimask[p, c] = 1 if p <= c  (k index on partitions, q on cols)
    in_maps = []
    for c in range(NCORES):
        h0 = 2 * c
        cols = np.r_[h0 * HD : (h0 + 2) * HD]
        wq = W_attn[:, cols]
        wk = W_attn[:, C + cols]
        wv = W_attn[:, 2 * C + cols]
        wqkv = np.concatenate([wq, wk, wv], axis=1).astype(bf16)
        bqkv = np.concatenate(
            [b_attn[cols], b_attn[C + cols], b_attn[2 * C + cols]]
        ).astype(np.float32)
        wproj = np.ascontiguousarray(W_proj[cols, :]).astype(bf16)
        in_maps.append(
            {
                "xtiles": xtiles,
                "wqkv": np.ascontiguousarray(wqkv),
                "bqkv": bqkv,
                "wproj": wproj,
                "trimask": np.ascontiguousarray(tri),
            }
        )
    return in_maps


def get_runner():
    global _RUNNER
    if _RUNNER is None:
        _RUNNER = Runner()
    return _RUNNER


def kernel(x, W_attn, b_attn, W_proj, b_proj):
    x = np.asarray(x, dtype=np.float32)
    W_attn = np.asarray(W_attn, dtype=np.float32)
    b_attn = np.asarray(b_attn, dtype=np.float32)
    W_proj = np.asarray(W_proj, dtype=np.float32)
    b_proj = np.asarray(b_proj, dtype=np.float32)
    runner = get_runner()
    in_maps = make_in_maps(x, W_attn, b_attn, W_proj, b_proj)
    results = runner.execute(in_maps)
    total = np.zeros((BT, C), np.float32)
    for r in results:
        total += np.asarray(r["out"], dtype=np.float32)
    total += b_proj[None, :]
    return total.reshape(B, T, C)
